# revision 25
# baseline (speedup 1.0000x reference)
"""ASTGCN block kernel for Trainium2 (8 NeuronCores, SPMD data-parallel over batch).

Strategy: the Chebyshev graph convolution (sum_k (cheb_k * S)^T @ xTheta_k,
a (1024 x 3072) @ (3072 x 4096) matmul per batch = ~85% of all FLOPs) runs
on-device via a Bass/Tile kernel in fp8-e4m3 with perf_mode=DoubleRow
(2 fp8 weights per PE cell -> 256-deep contraction per matmul instruction),
data-parallel over batch across the 8 cores (2 batches/core).  The graph-conv
branch feeds the output through a residual connection whose exact (host fp32)
path is ~1000x larger in magnitude, so fp8 quantization error on this matmul
is ~1e-4 of the final output (gate is 2e-2).  The small attention matmuls
(producing S) and the elementwise epilogue run on host in fp32.
"""

import numpy as np
import ml_dtypes
from contextlib import ExitStack

import concourse.bass as bass
import concourse.bacc as bacc
import concourse.mybir as mybir
from concourse import tile
from concourse.bass_utils import run_bass_kernel_spmd

B, N, Fh, T, K, C = 16, 1024, 64, 64, 3, 64
NCORES = 8
BLOC = B // NCORES  # 2 batches per core
KM = K * N          # 3072 contraction length
CT = C * T          # 4096 output free length

NKC = KM // 256     # 12 contraction chunks of 256 (2x128 DoubleRow pairs)
NXC = CT // 1024    # 4 column chunks of 1024
NNT = N // 128      # 8 output row tiles

_cached = {}
_last_in_maps = None


def _build_nc():
    if "nc" in _cached:
        return _cached["nc"]
    nc = bacc.Bacc("TRN2", target_bir_lowering=False, debug=False)
    fp8 = mybir.dt.float8e4
    bf16 = mybir.dt.bfloat16
    f32 = mybir.dt.float32
    DR = mybir.MatmulPerfMode.DoubleRow

    # A laid out on host as [b, p, kc, j, n] so each per-batch load is one
    # DMA with fully contiguous 24.5KB-per-partition payload.
    a_d = nc.declare_dram_parameter("A", [BLOC, 128, NKC, 2, N], fp8, isOutput=False)
    # XT laid out as [b, xc, p, kc, j, 1024].
    x_d = nc.declare_dram_parameter(
        "XT", [BLOC, NXC, 128, NKC, 2, 1024], fp8, isOutput=False
    )
    o_d = nc.declare_dram_parameter("OUT", [BLOC, N, CT], bf16, isOutput=True)

    with tile.TileContext(nc) as tc, ExitStack() as ctx:
        apool = ctx.enter_context(tc.tile_pool(name="a", bufs=2))
        xpool = ctx.enter_context(tc.tile_pool(name="x", bufs=4))
        opool = ctx.enter_context(tc.tile_pool(name="o", bufs=8))
        tpool = ctx.enter_context(tc.tile_pool(name="t", bufs=32))
        pspool = ctx.enter_context(
            tc.tile_pool(name="ps", bufs=8, space=bass.MemorySpace.PSUM)
        )
        asbs = {}
        xsbs = {}

        def emit_a_load(b):
            asb = apool.tile([128, NKC, 2, N], fp8, tag="a", name="asb")
            nc.sync.dma_start(asb[:], a_d[b])
            asbs[b] = asb

        def emit_x_load(b, xc, split=False):
            xsb = xpool.tile([128, NKC, 2, 1024], fp8, tag="x", name="xsb")
            if split:
                hk = NKC // 2
                nc.sync.dma_start(xsb[:, 0:hk], x_d[b, xc, :, 0:hk])
                nc.sync.dma_start(xsb[:, hk:NKC], x_d[b, xc, :, hk:NKC])
            else:
                nc.sync.dma_start(xsb[:], x_d[b, xc])
            xsbs[(b, xc)] = xsb

        # Pipeline-fill: A0 and X00 are on the PE's critical path, so they
        # are emitted as per-kc slices, interleaved (A[kc], Xa[kc]) so that
        # accumulation can start as soon as the first slices land.
        asb0 = apool.tile([128, NKC, 2, N], fp8, tag="a", name="asb0")
        xsb00 = xpool.tile([128, NKC, 2, 1024], fp8, tag="x", name="xsb00")
        for kc in range(NKC):
            nc.sync.dma_start(asb0[:, kc], a_d[0, :, kc])
            nc.sync.dma_start(xsb00[:, kc], x_d[0, 0, :, kc])
        asbs[0] = asb0
        xsbs[(0, 0)] = xsb00

        def compute_head(b, xc):
            # fill-phase chunk: split the 12-kc accumulation into two 6-kc
            # sub-groups so PSUM banks retire halfway through the head DMA
            # stream, doubling the matmul work exposed while loads trickle
            # in.  Sub-results are drained to bf16 temps and combined.
            asb = asbs[b]
            xsb = xsbs[(b, xc)]
            HK = NKC // 2
            tmps = {}
            for half in range(2):
                for nt in range(NNT):
                    for h in range(2):
                        ps = pspool.tile([128, 512], f32, tag="ps",
                                         name="psf")
                        for kc in range(half * HK, (half + 1) * HK):
                            nc.tensor.matmul(
                                ps[:],
                                asb[:, kc, :, nt * 128 : (nt + 1) * 128],
                                xsb[:, kc, :, h * 512 : (h + 1) * 512],
                                start=(kc == half * HK),
                                stop=(kc == (half + 1) * HK - 1),
                                perf_mode=DR,
                            )
                        t = tpool.tile([128, 512], bf16, tag="t", name="tmp")
                        eng = nc.vector.tensor_copy if (nt + h + half) % 2 \
                            else nc.scalar.copy
                        eng(t[:], ps[:])
                        tmps[(nt, h, half)] = t
            for nt in range(NNT):
                osb = opool.tile([128, 1024], bf16, tag="o", name="osb_h")
                for h in range(2):
                    nc.vector.tensor_add(
                        osb[:, h * 512 : (h + 1) * 512],
                        tmps[(nt, h, 0)][:], tmps[(nt, h, 1)][:],
                    )
                nc.sync.dma_start(
                    o_d[b, nt * 128 : (nt + 1) * 128,
                        xc * 1024 : (xc + 1) * 1024],
                    osb[:],
                )

        def compute_chunk(b, xc, last=False):
            asb = asbs[b]
            xsb = xsbs[(b, xc)]
            for nt in range(NNT):
                tail = last and nt == NNT - 1
                osb = opool.tile([128, 1024], bf16, tag="o", name="osb")
                ps0 = pspool.tile([128, 512], f32, tag="ps", name="ps0")
                ps1 = pspool.tile([128, 512], f32, tag="ps", name="ps1")
                if tail:
                    # h-sequential so the first half's drain+store chain
                    # overlaps the second half's matmuls
                    for h, ps in ((0, ps0), (1, ps1)):
                        for kc in range(NKC):
                            nc.tensor.matmul(
                                ps[:],
                                asb[:, kc, :, nt * 128 : (nt + 1) * 128],
                                xsb[:, kc, :, h * 512 : (h + 1) * 512],
                                start=(kc == 0), stop=(kc == NKC - 1),
                                perf_mode=DR,
                            )
                        eng = nc.vector.tensor_copy if h else nc.scalar.copy
                        eng(osb[:, h * 512 : (h + 1) * 512], ps[:])
                        nc.sync.dma_start(
                            o_d[b, nt * 128 : (nt + 1) * 128,
                                xc * 1024 + h * 512 : xc * 1024 + (h + 1) * 512],
                            osb[:, h * 512 : (h + 1) * 512],
                        )
                    continue
                for kc in range(NKC):
                    st = kc == 0
                    sp = kc == NKC - 1
                    lhsT = asb[:, kc, :, nt * 128 : (nt + 1) * 128]
                    nc.tensor.matmul(
                        ps0[:], lhsT, xsb[:, kc, :, 0:512],
                        start=st, stop=sp, perf_mode=DR,
                    )
                    nc.tensor.matmul(
                        ps1[:], lhsT, xsb[:, kc, :, 512:1024],
                        start=st, stop=sp, perf_mode=DR,
                    )
                if nt % 2:
                    nc.vector.tensor_copy(osb[:, 0:512], ps0[:])
                    nc.scalar.copy(osb[:, 512:1024], ps1[:])
                else:
                    nc.scalar.copy(osb[:, 0:512], ps0[:])
                    nc.vector.tensor_copy(osb[:, 512:1024], ps1[:])
                nc.sync.dma_start(
                    o_d[b, nt * 128 : (nt + 1) * 128,
                        xc * 1024 : (xc + 1) * 1024],
                    osb[:],
                )

        # software-pipelined emission: each chunk's load is emitted one
        # compute-chunk ahead of its use; A1 two chunks ahead
        jobs = [(b, xc) for b in range(BLOC) for xc in range(NXC)]
        for i, (b, xc) in enumerate(jobs):
            if i + 1 < len(jobs):
                nb, nxc = jobs[i + 1]
                if nxc == 0:
                    emit_a_load(nb)
                emit_x_load(nb, nxc, split=(i == 0))
            if i <= 1:
                compute_head(b, xc)
            else:
                compute_chunk(b, xc, last=(i == len(jobs) - 1))
    nc.compile()
    _cached["nc"] = nc
    return nc


def _softmax_ax1(s):
    m = s.max(axis=1, keepdims=True)
    e = np.exp(s - m)
    return e / e.sum(axis=1, keepdims=True)


def _sigmoid(x):
    return 1.0 / (1.0 + np.exp(-x))


def _pow2_scale(absmax):
    # scale to land absmax in (90, 180]; fp8e4 (TRN) saturates at 240
    return 2.0 ** np.floor(np.log2(180.0 / max(absmax, 1e-30)))


def kernel(x, cheb, Theta, W1, W2, W3, b_s, V_s, U1, U2, U3, b_e, V_e,
           tw, tb, rw, rb, gamma, beta):
    f32 = np.float32
    x = np.asarray(x, f32)
    cheb = np.asarray(cheb, f32)
    Theta = np.asarray(Theta, f32)

    # ---- temporal attention (host, small) ----
    xu1 = np.einsum("bnft,n->btf", x, np.asarray(U1, f32), optimize=True)  # (B,T,F)
    lhs = xu1 @ np.asarray(U2, f32)                                        # (B,T,N)
    rhs = np.einsum("f,bnft->bnt", np.asarray(U3, f32), x, optimize=True)  # (B,N,T)
    prod = np.matmul(lhs, rhs)                                             # (B,T,T)
    sig = _sigmoid(prod + np.asarray(b_e, f32))
    E = np.einsum("kj,bij->bik", np.asarray(V_e, f32), sig, optimize=True)
    E = _softmax_ax1(E)
    x_tat = np.einsum("bnfj,bjt->bnft", x, E, optimize=True)

    # ---- spatial attention (host) ----
    lhs2 = np.einsum("bnft,t->bnf", x_tat, np.asarray(W1, f32), optimize=True) @ \
        np.asarray(W2, f32)                                                # (B,N,T)
    rhs2 = np.einsum("f,bnft->btn", np.asarray(W3, f32), x_tat, optimize=True)
    prod2 = np.matmul(lhs2, rhs2)                                          # (B,N,N)
    sig2 = _sigmoid(prod2 + np.asarray(b_s, f32))
    S = np.einsum("kj,bij->bik", np.asarray(V_s, f32), sig2, optimize=True)
    S = _softmax_ax1(S)                                                    # (B,N,N)

    # ---- device operands for the big graph-conv matmul ----
    # A[b,(k,m),n] = cheb[k,m,n] * S[b,m,n]
    A = (cheb[None, :, :, :] * S[:, None, :, :]).reshape(B, KM, N)
    # XT[b,(k,m),(o,t)] = sum_f x[b,m,f,t] * Theta[k,f,o]
    xbtf = np.ascontiguousarray(x.transpose(0, 1, 3, 2)).reshape(B * N * T, Fh)
    xt_parts = []
    for k in range(K):
        p = (xbtf @ Theta[k]).reshape(B, N, T, C).transpose(0, 1, 3, 2)  # (B,N,C,T)
        xt_parts.append(p.reshape(B, N, CT))
    XT = np.stack(xt_parts, axis=1).reshape(B, KM, CT)

    # fp8 packing with power-of-2 scaling (contraction r = kc*256 + j*128 + p)
    sA = _pow2_scale(np.abs(A).max())
    sX = _pow2_scale(np.abs(XT).max())
    fp8 = ml_dtypes.float8_e4m3
    # A: (B, KM, N) -> (B, p, kc, j, n)
    A8 = np.ascontiguousarray(
        (A * sA).reshape(B, NKC, 2, 128, N).transpose(0, 3, 1, 2, 4)
    ).astype(fp8)
    # XT: (B, KM, CT) -> (B, xc, p, kc, j, 1024)
    X8 = np.ascontiguousarray(
        (XT * sX).reshape(B, NKC, 2, 128, NXC, 1024).transpose(0, 4, 3, 1, 2, 5)
    ).astype(fp8)

    nc = _build_nc()
    in_maps = [
        {"A": A8[c * BLOC : (c + 1) * BLOC], "XT": X8[c * BLOC : (c + 1) * BLOC]}
        for c in range(NCORES)
    ]
    global _last_in_maps
    _last_in_maps = in_maps
    res = run_bass_kernel_spmd(nc, in_maps, core_ids=list(range(NCORES)))
    out = np.concatenate([r["OUT"] for r in res.results], axis=0)  # (B,N,CT) bf16
    out = out.astype(f32) / (sA * sX)
    out = out.reshape(B, N, C, T)

    # ---- epilogue (host): relu, temporal conv, residual, layernorm ----
    sg = np.maximum(out, 0.0)
    tw = np.asarray(tw, f32)  # (C, C, 1, 3) OIHW
    tc_out = np.einsum("bnit,oi->bnot", sg, tw[:, :, 0, 1], optimize=True)
    t0 = np.einsum("bnit,oi->bnot", sg[:, :, :, :-1], tw[:, :, 0, 0], optimize=True)
    tc_out[:, :, :, 1:] += t0
    t2 = np.einsum("bnit,oi->bnot", sg[:, :, :, 1:], tw[:, :, 0, 2], optimize=True)
    tc_out[:, :, :, :-1] += t2
    tc_out += np.asarray(tb, f32)[None, None, :, None]

    res_c = np.einsum("bnft,of->bnot", x, np.asarray(rw, f32), optimize=True)
    res_c += np.asarray(rb, f32)[None, None, :, None]

    y = np.maximum(res_c + tc_out, 0.0)
    mu = y.mean(axis=-1, keepdims=True)
    var = y.var(axis=-1, keepdims=True)
    yn = (y - mu) / np.sqrt(var + 1e-5)
    # gamma/beta are (64,) and broadcast along the LAST axis (T), as in reference
    yn = yn * np.asarray(gamma, f32) + np.asarray(beta, f32)
    return yn.astype(np.float32)
